# revision 24
# baseline (speedup 1.0000x reference)
"""ChebConv (K=3) kernel for Trainium2, data-parallel over batch across 8 NeuronCores.

Math (per batch b):
    d    = adj.sum(axis=1)  (row sums), dinv = (d+eps)^-0.5, dsq = (d+eps)^0.5
    M    = Dinv A Dinv  (L = I - M)
    Tx0 = x, Tx1 = L x, Tx2 = 2 L Tx1 - Tx0
    out  = relu(sum_k Txk @ W[k] + sum_k b[k])

Kernel-side reformulation with u_k := Dsq Tx_k and UNSCALED at2[j, i] = A[i, j]:
every PE pass contracts a Dinv^2-scaled natural operand against plain A^T, so
the transpose of A needs no scaling (it only depends on the bf16 cast, not the
row-sum chain) and the Chebyshev recurrence is elementwise in the transposed
domain:
    u0  = Dsq x,  yp0 = Dinv x  (= Dinv^2 u0)
    z1T = pass(yp0) : z1T[f,i] = sum_j yp0[j,f] at2[j,i]   ( = (Dsq M x)^T )
    u1T = u0T - z1T                                  (DVE subtract)
    y1s[c] = dinv2[c] * u1n[c]   (scale fused into the PSUM->SBUF copy)
    z2T = pass(y1s)                                  ( = (Dsq M Tx1)^T )
    u2T = 2 u1T - u0T - 2 z2T                        (DVE)
    out = relu(Dinv @ (sum_k u_k @ W[k]) + bsum)     (lhsT = u_kT blocks)

A row-strips stream as fp32 in two half-strip DMAs; DVE casts+row-sums the low
half while ACT does the high half (fused accum_out). Transposition of A is
split: blocks c=0..7 on the PE (matmul vs identity, PSUM->SBUF copies on
DVE/ACT), blocks c=8..15 via the DMA transpose XBAR (SBUF->SBUF bf16, scalar
ring) -- this offloads half the pool-engine copy traffic to the otherwise
idle DMA headroom. Pass 1 is emitted triangularly, one strip behind the load
loop, so the PE queue always has cast-ready transpose work at its head.
"""

import numpy as np

B, N, F, K = 8, 2048, 128, 3
P = 128
NT = N // P  # 16
H = N // 2   # 1024
NXB = 8      # A^T blocks per strip produced by the DMA transpose XBAR
EPS = 1e-6
NCORES = 8

_cache = {}


def _build_nc():
    from contextlib import ExitStack

    import concourse.bacc as bacc
    import concourse.tile as tile
    from concourse import mybir

    f32 = mybir.dt.float32
    bf16 = mybir.dt.bfloat16
    AF = mybir.ActivationFunctionType
    OP = mybir.AluOpType

    nc = bacc.Bacc("TRN2", target_bir_lowering=False, debug=False, num_devices=NCORES)
    adj = nc.dram_tensor("adj", [N, N], f32, kind="ExternalInput").ap()
    x = nc.dram_tensor("x", [N, F], f32, kind="ExternalInput").ap()
    wb_d = nc.dram_tensor("wb", [P, K, F], bf16, kind="ExternalInput").ap()
    bsum_d = nc.dram_tensor("bsum", [P, F], f32, kind="ExternalInput").ap()
    identb_d = nc.dram_tensor("identb", [P, P], bf16, kind="ExternalInput").ap()
    out = nc.dram_tensor("out", [N, F], f32, kind="ExternalOutput").ap()
    out_t = out.rearrange("(t p) f -> p t f", p=P)

    NPE = NT - NXB  # PE-transposed blocks per strip (c = 0..NPE-1)

    with ExitStack() as ctx:
        tc = ctx.enter_context(tile.TileContext(nc))
        consts = ctx.enter_context(tc.tile_pool(name="consts", bufs=1))
        afp = ctx.enter_context(tc.tile_pool(name="afp", bufs=5))
        abp = ctx.enter_context(tc.tile_pool(name="abp", bufs=4))
        xfp = ctx.enter_context(tc.tile_pool(name="xfp", bufs=3))
        big = ctx.enter_context(tc.tile_pool(name="big", bufs=1))
        small = ctx.enter_context(tc.tile_pool(name="small", bufs=3))
        ps_acc = ctx.enter_context(tc.tile_pool(name="ps_acc", bufs=1, space="PSUM"))
        ps_t = ctx.enter_context(tc.tile_pool(name="ps_t", bufs=4, space="PSUM"))

        # ---- constants (all HWDGE, scalar ring; sync ring = adj + out) --
        ident_bf = consts.tile([P, P], bf16)
        nc.scalar.dma_start(out=ident_bf, in_=identb_d)
        w_bf = consts.tile([P, K, F], bf16)
        nc.scalar.dma_start(out=w_bf, in_=wb_d)
        bsum = consts.tile([P, F], f32)
        nc.scalar.dma_start(out=bsum, in_=bsum_d)
        eps_sb = consts.tile([P, 1], f32)
        nc.vector.memset(eps_sb, EPS)

        # per-node scalars, [P, NT]: column r holds values for node tile r
        dinv = consts.tile([P, NT], f32)
        dinv2 = consts.tile([P, NT], f32)

        u0 = big.tile([P, NT, F], bf16)    # Dsq x, natural (u0T source)
        yp0 = big.tile([P, NT, F], bf16)   # Dinv x, natural (pass-1 lhsT)
        y1s = big.tile([P, NT, F], bf16)   # dinv2 * u1, natural (pass-2 lhsT)
        at2 = big.tile([P, NT, N], bf16)   # [j_in_tile, c(j tile), i]: A[i,j]
        uT0 = big.tile([P, N], bf16)       # transposed u0: [f, i]
        uT1 = big.tile([P, N], bf16)
        uT2 = big.tile([P, N], bf16)
        ttT = big.tile([P, N], bf16)       # 2*u1T - u0T

        z1 = ps_acc.tile([P, N], f32, tag="acc")

        def emit_u0T_and_pass1(r):
            # u0T strip r (transpose u0 via PE)
            pt0 = ps_t.tile([P, 4, P], f32, tag="t")
            nc.tensor.matmul(pt0[:, 0, :], lhsT=u0[:, r, :], rhs=ident_bf,
                             start=True, stop=True)
            if r % 2 == 0:
                nc.vector.tensor_copy(out=uT0[:, r * P:(r + 1) * P],
                                      in_=pt0[:, 0, :])
            else:
                nc.scalar.copy(out=uT0[:, r * P:(r + 1) * P], in_=pt0[:, 0, :])
            # triangular pass-1 terms that became ready with strip r:
            # (a) older strips s < r with new weight block c = r (bank chunks)
            for sg in range((r + 3) // 4):
                lo = 4 * sg
                hi = min(lo + 4, r)  # strips [lo, hi)
                nc.tensor.matmul(z1[:, lo * P:hi * P], lhsT=yp0[:, r, :],
                                 rhs=at2[:, r, lo * P:hi * P],
                                 start=False, stop=(r == NT - 1),
                                 skip_group_check=True)
            # (b) strip r, weight blocks c <= r
            for c in range(r + 1):
                nc.tensor.matmul(z1[:, r * P:(r + 1) * P], lhsT=yp0[:, c, :],
                                 rhs=at2[:, c, r * P:(r + 1) * P],
                                 start=(r % 4 == 0 and c == 0),
                                 stop=(c == NT - 1), skip_group_check=True)

        # ---- streaming phase -------------------------------------------
        for r in range(NT):
            a_f = afp.tile([P, N], f32, tag="af")
            nc.sync.dma_start(out=a_f[:, :H], in_=adj[r * P:(r + 1) * P, :H])
            nc.sync.dma_start(out=a_f[:, H:], in_=adj[r * P:(r + 1) * P, H:])
            x_t = xfp.tile([P, F], f32, tag="x")
            nc.scalar.dma_start(out=x_t, in_=x[r * P:(r + 1) * P, :])

            # fused fp32->bf16 cast + row-sum halves (DVE low, ACT high)
            a_t = abp.tile([P, N], bf16, tag="a")
            d_lo = small.tile([P, 1], f32, tag="dlo")
            d_hi = small.tile([P, 1], f32, tag="dhi")
            nc.vector.tensor_scalar(
                out=a_t[:, :H], in0=a_f[:, :H], scalar1=1.0, scalar2=0.0,
                op0=OP.mult, op1=OP.add, accum_out=d_lo)
            nc.scalar.activation(out=a_t[:, H:], in_=a_f[:, H:],
                                 func=AF.Identity, accum_out=d_hi)

            # transpose A strip r, blocks 0..NPE-1 on the PE
            for g in range(NPE // 4):
                pt = ps_t.tile([P, 4, P], f32, tag="t")
                for q in range(4):
                    c = 4 * g + q
                    nc.tensor.matmul(pt[:, q, :], lhsT=a_t[:, c * P:(c + 1) * P],
                                     rhs=ident_bf, start=True, stop=True)
                if g % 2 == 0:
                    nc.vector.tensor_copy(
                        out=at2[:, 4 * g:4 * g + 4, r * P:(r + 1) * P], in_=pt)
                else:
                    nc.scalar.copy(
                        out=at2[:, 4 * g:4 * g + 4, r * P:(r + 1) * P], in_=pt)
            # blocks NPE..NT-1 via the DMA transpose XBAR (SBUF->SBUF bf16)
            for c in range(NPE, NT):
                nc.scalar.dma_start(
                    out=at2[:, c, r * P:(r + 1) * P],
                    in_=a_t[:, c * P:(c + 1) * P], transpose=True)

            # scalar chain for strip r
            d_r = small.tile([P, 1], f32, tag="d")
            nc.vector.tensor_scalar(out=d_r, in0=d_lo, scalar1=d_hi,
                                    scalar2=None, op0=OP.add)
            dsq_r = small.tile([P, 1], f32, tag="dsq")
            nc.scalar.activation(out=dsq_r, in_=d_r, func=AF.Sqrt, bias=eps_sb)
            nc.vector.reciprocal(out=dinv[:, r:r + 1], in_=dsq_r)
            nc.vector.tensor_scalar(out=u0[:, r, :], in0=x_t,
                                    scalar1=dsq_r, scalar2=None, op0=OP.mult)
            nc.vector.tensor_scalar(out=yp0[:, r, :], in0=x_t,
                                    scalar1=dinv[:, r:r + 1], scalar2=None,
                                    op0=OP.mult)

            # PE work that depends on strip r-1's scalar chain, emitted here
            # so that strip r's transposes (ready as soon as the cast lands)
            # sit AHEAD of it in the PE queue -- avoids head-of-line stalls.
            if r >= 1:
                emit_u0T_and_pass1(r - 1)
        emit_u0T_and_pass1(NT - 1)

        # dinv2 = dinv*dinv, all 16 strips in one bulk op
        nc.vector.scalar_tensor_tensor(out=dinv2, in0=dinv, scalar=1.0,
                                       in1=dinv, op0=OP.mult, op1=OP.mult)

        # ---- recurrence: u1T = u0T - z1T (pure DVE, no scaling) --------
        for ch in range(4):
            s = slice(ch * 512, (ch + 1) * 512)
            nc.vector.scalar_tensor_tensor(
                out=uT1[:, s], in0=z1[:, s], scalar=-1.0, in1=uT0[:, s],
                op0=OP.mult, op1=OP.add)

        # ---- pass 2: y1s[c] = dinv2[c]*u1n[c] (scale fused into the
        #      PSUM->SBUF copy, alternating DVE/ACT), then wide MMs ------
        z2 = ps_acc.tile([P, N], f32, tag="acc")
        for c in range(NT):
            ptc = ps_t.tile([P, 4, P], f32, tag="t")
            nc.tensor.matmul(ptc[:, 0, :], lhsT=uT1[:, c * P:(c + 1) * P],
                             rhs=ident_bf, start=True, stop=True)
            if c % 2 == 0:
                nc.vector.tensor_scalar(out=y1s[:, c, :], in0=ptc[:, 0, :],
                                        scalar1=dinv2[:, c:c + 1],
                                        scalar2=None, op0=OP.mult)
            else:
                nc.scalar.mul(out=y1s[:, c, :], in_=ptc[:, 0, :],
                              mul=dinv2[:, c:c + 1])
            for nch in range(4):
                nc.tensor.matmul(z2[:, nch * 512:(nch + 1) * 512],
                                 lhsT=y1s[:, c, :],
                                 rhs=at2[:, c, nch * 512:(nch + 1) * 512],
                                 start=(c == 0), stop=(c == NT - 1))

        # ttT = 2*u1T - u0T (runs on DVE while PE grinds pass 2)
        for ch in range(4):
            s = slice(ch * 512, (ch + 1) * 512)
            nc.vector.scalar_tensor_tensor(
                out=ttT[:, s], in0=uT1[:, s], scalar=2.0, in1=uT0[:, s],
                op0=OP.mult, op1=OP.subtract)

        # ---- output layer: out[i,:] = relu(dinv[i]*(sum_k uTk[:,i]^T Wk)+bsum)
        #      u2T = ttT - 2*z2T produced chunk-by-chunk, interleaved. ----
        uTs = (uT0, uT1, uT2)
        for g in range(4):
            s = slice(g * 512, (g + 1) * 512)
            nc.vector.scalar_tensor_tensor(
                out=uT2[:, s], in0=z2[:, s], scalar=-2.0, in1=ttT[:, s],
                op0=OP.mult, op1=OP.add)
            og = small.tile([P, 4, F], f32, tag="og")
            for q in range(4):
                rr = 4 * g + q
                oc = ps_t.tile([P, 4, P], f32, tag="t")
                for k3 in range(K):
                    nc.tensor.matmul(oc[:, 0, :],
                                     lhsT=uTs[k3][:, rr * P:(rr + 1) * P],
                                     rhs=w_bf[:, k3, :],
                                     start=(k3 == 0), stop=(k3 == K - 1))
                tmp = small.tile([P, F], f32, tag="tmp")
                nc.vector.scalar_tensor_tensor(
                    out=tmp, in0=oc[:, 0, :], scalar=dinv[:, rr:rr + 1],
                    in1=bsum, op0=OP.mult, op1=OP.add)
                nc.scalar.activation(out=og[:, q, :], in_=tmp, func=AF.Relu)
            nc.sync.dma_start(out=out_t[:, 4 * g:4 * g + 4, :], in_=og)

    nc.compile()
    return nc


def _get_nc():
    if "nc" not in _cache:
        _cache["nc"] = _build_nc()
    return _cache["nc"]


def make_in_maps(x, adj, W, b):
    import ml_dtypes

    bf16 = ml_dtypes.bfloat16
    identb = np.ascontiguousarray(np.eye(P, dtype=np.float32).astype(bf16))
    x = np.ascontiguousarray(np.asarray(x, dtype=np.float32))
    adj = np.ascontiguousarray(np.asarray(adj, dtype=np.float32))
    # W [K, in, out] -> [in, K, out] bf16 (host-side rearrange + cast)
    wb = np.ascontiguousarray(
        np.asarray(W, dtype=np.float32).transpose(1, 0, 2).astype(bf16))
    bf = np.asarray(b, dtype=np.float32)
    bsum = np.ascontiguousarray(
        np.broadcast_to(bf.sum(axis=0), (P, F)).astype(np.float32))
    return [
        {"adj": adj[c], "x": x[c], "wb": wb, "bsum": bsum, "identb": identb}
        for c in range(NCORES)
    ]


def run_raw(x, adj, W, b, **kwargs):
    from concourse import bass_utils

    nc = _get_nc()
    in_maps = make_in_maps(x, adj, W, b)
    res = bass_utils.run_bass_kernel_spmd(nc, in_maps,
                                          core_ids=list(range(NCORES)), **kwargs)
    out = np.stack([res.results[c]["out"] for c in range(NCORES)], axis=0)
    return out.astype(np.float32), res


def kernel(x, adj, W, b):
    out, _ = run_raw(x, adj, W, b)
    return out


# revision 25
# speedup vs baseline: 2.9138x; 2.9138x over previous
"""ChebConv (K=3) kernel for Trainium2, data-parallel over batch across 8 NeuronCores.

Math (per batch b):
    d    = adj.sum(axis=1)  (row sums), dinv = (d+eps)^-0.5, dsq = (d+eps)^0.5
    M    = Dinv A Dinv  (L = I - M)
    Tx0 = x, Tx1 = L x, Tx2 = 2 L Tx1 - Tx0
    out  = relu(sum_k Txk @ W[k] + sum_k b[k])

Kernel-side reformulation with u_k := Dsq Tx_k and UNSCALED at2[j, i] = A[i, j]:
every PE pass contracts a Dinv^2-scaled natural operand against plain A^T, so
the transpose of A needs no scaling (it only depends on the bf16 cast, not the
row-sum chain) and the Chebyshev recurrence is elementwise in the transposed
domain:
    u0  = Dsq x,  yp0 = Dinv x  (= Dinv^2 u0)
    z1T = pass(yp0) : z1T[f,i] = sum_j yp0[j,f] at2[j,i]   ( = (Dsq M x)^T )
    u1T = u0T - z1T                                  (DVE subtract)
    y1s[c] = dinv2[c] * u1n[c]   (scale fused into the PSUM->SBUF copy)
    z2T = pass(y1s)                                  ( = (Dsq M Tx1)^T )
    u2T = 2 u1T - u0T - 2 z2T                        (DVE)
    out = relu(Dinv @ (sum_k u_k @ W[k]) + bsum)     (lhsT = u_kT blocks)

A row-strips stream as fp32 in two half-strip DMAs; DVE casts+row-sums the low
half while ACT does the high half (fused accum_out). Transposition of A is
split: blocks c=0..7 on the PE (matmul vs identity, PSUM->SBUF copies on
DVE/ACT), blocks c=8..15 via the DMA transpose XBAR (SBUF->SBUF bf16, scalar
ring) -- this offloads half the pool-engine copy traffic to the otherwise
idle DMA headroom. Pass 1 is emitted triangularly, one strip behind the load
loop, so the PE queue always has cast-ready transpose work at its head.
"""

import numpy as np

B, N, F, K = 8, 2048, 128, 3
P = 128
NT = N // P  # 16
H = N // 2   # 1024
NXB = 0      # A^T blocks per strip produced by the DMA transpose XBAR
EPS = 1e-6
NCORES = 8

_cache = {}


def _build_nc():
    from contextlib import ExitStack

    import concourse.bacc as bacc
    import concourse.tile as tile
    from concourse import mybir

    f32 = mybir.dt.float32
    bf16 = mybir.dt.bfloat16
    AF = mybir.ActivationFunctionType
    OP = mybir.AluOpType

    nc = bacc.Bacc("TRN2", target_bir_lowering=False, debug=False, num_devices=NCORES)
    adj = nc.dram_tensor("adj", [N, N], f32, kind="ExternalInput").ap()
    x = nc.dram_tensor("x", [N, F], f32, kind="ExternalInput").ap()
    wb_d = nc.dram_tensor("wb", [P, K, F], bf16, kind="ExternalInput").ap()
    bsum_d = nc.dram_tensor("bsum", [P, F], f32, kind="ExternalInput").ap()
    identb_d = nc.dram_tensor("identb", [P, P], bf16, kind="ExternalInput").ap()
    out = nc.dram_tensor("out", [N, F], f32, kind="ExternalOutput").ap()
    out_t = out.rearrange("(t p) f -> p t f", p=P)

    NPE = NT - NXB  # PE-transposed blocks per strip (c = 0..NPE-1)

    with ExitStack() as ctx:
        tc = ctx.enter_context(tile.TileContext(nc))
        consts = ctx.enter_context(tc.tile_pool(name="consts", bufs=1))
        afp = ctx.enter_context(tc.tile_pool(name="afp", bufs=5))
        abp = ctx.enter_context(tc.tile_pool(name="abp", bufs=4))
        xfp = ctx.enter_context(tc.tile_pool(name="xfp", bufs=3))
        big = ctx.enter_context(tc.tile_pool(name="big", bufs=1))
        small = ctx.enter_context(tc.tile_pool(name="small", bufs=3))
        ps_acc = ctx.enter_context(tc.tile_pool(name="ps_acc", bufs=1, space="PSUM"))
        ps_t = ctx.enter_context(tc.tile_pool(name="ps_t", bufs=4, space="PSUM"))

        # ---- constants (all HWDGE, scalar ring; sync ring = adj + out) --
        ident_bf = consts.tile([P, P], bf16)
        nc.sync.dma_start(out=ident_bf, in_=identb_d)
        w_bf = consts.tile([P, K, F], bf16)
        nc.sync.dma_start(out=w_bf, in_=wb_d)
        bsum = consts.tile([P, F], f32)
        nc.scalar.dma_start(out=bsum, in_=bsum_d)
        eps_sb = consts.tile([P, 1], f32)
        nc.vector.memset(eps_sb, EPS)

        # per-node scalars, [P, NT]: column r holds values for node tile r
        dinv = consts.tile([P, NT], f32)
        dinv2 = consts.tile([P, NT], f32)

        u0 = big.tile([P, NT, F], bf16)    # Dsq x, natural (u0T source)
        yp0 = big.tile([P, NT, F], bf16)   # Dinv x, natural (pass-1 lhsT)
        y1s = big.tile([P, NT, F], bf16)   # dinv2 * u1, natural (pass-2 lhsT)
        at2 = big.tile([P, NT, N], bf16)   # [j_in_tile, c(j tile), i]: A[i,j]
        uT0 = big.tile([P, N], bf16)       # transposed u0: [f, i]
        uT1 = big.tile([P, N], bf16)
        uT2 = big.tile([P, N], bf16)
        ttT = big.tile([P, N], bf16)       # 2*u1T - u0T

        z1 = ps_acc.tile([P, N], f32, tag="acc")

        def emit_u0T_and_pass1(r):
            # u0T strip r (transpose u0 via PE)
            pt0 = ps_t.tile([P, 4, P], f32, tag="t")
            nc.tensor.matmul(pt0[:, 0, :], lhsT=u0[:, r, :], rhs=ident_bf,
                             start=True, stop=True)
            if r % 2 == 0:
                nc.vector.tensor_copy(out=uT0[:, r * P:(r + 1) * P],
                                      in_=pt0[:, 0, :])
            else:
                nc.scalar.copy(out=uT0[:, r * P:(r + 1) * P], in_=pt0[:, 0, :])
            # triangular pass-1 terms that became ready with strip r:
            # (a) older strips s < r with new weight block c = r (bank chunks)
            for sg in range((r + 3) // 4):
                lo = 4 * sg
                hi = min(lo + 4, r)  # strips [lo, hi)
                nc.tensor.matmul(z1[:, lo * P:hi * P], lhsT=yp0[:, r, :],
                                 rhs=at2[:, r, lo * P:hi * P],
                                 start=False, stop=(r == NT - 1),
                                 skip_group_check=True)
            # (b) strip r, weight blocks c <= r
            for c in range(r + 1):
                nc.tensor.matmul(z1[:, r * P:(r + 1) * P], lhsT=yp0[:, c, :],
                                 rhs=at2[:, c, r * P:(r + 1) * P],
                                 start=(r % 4 == 0 and c == 0),
                                 stop=(c == NT - 1), skip_group_check=True)

        # ---- streaming phase -------------------------------------------
        for r in range(NT):
            a_f = afp.tile([P, N], f32, tag="af")
            nc.sync.dma_start(out=a_f[:, :H], in_=adj[r * P:(r + 1) * P, :H])
            nc.sync.dma_start(out=a_f[:, H:], in_=adj[r * P:(r + 1) * P, H:])
            x_t = xfp.tile([P, F], f32, tag="x")
            nc.scalar.dma_start(out=x_t, in_=x[r * P:(r + 1) * P, :])

            # fused fp32->bf16 cast + row-sum halves (DVE low, ACT high)
            a_t = abp.tile([P, N], bf16, tag="a")
            d_lo = small.tile([P, 1], f32, tag="dlo")
            d_hi = small.tile([P, 1], f32, tag="dhi")
            nc.vector.tensor_scalar(
                out=a_t[:, :H], in0=a_f[:, :H], scalar1=1.0, scalar2=0.0,
                op0=OP.mult, op1=OP.add, accum_out=d_lo)
            nc.scalar.activation(out=a_t[:, H:], in_=a_f[:, H:],
                                 func=AF.Identity, accum_out=d_hi)

            # transpose A strip r, blocks 0..NPE-1 on the PE
            for g in range(NPE // 4):
                pt = ps_t.tile([P, 4, P], f32, tag="t")
                for q in range(4):
                    c = 4 * g + q
                    nc.tensor.matmul(pt[:, q, :], lhsT=a_t[:, c * P:(c + 1) * P],
                                     rhs=ident_bf, start=True, stop=True)
                if g % 2 == 0:
                    nc.vector.tensor_copy(
                        out=at2[:, 4 * g:4 * g + 4, r * P:(r + 1) * P], in_=pt)
                else:
                    nc.scalar.copy(
                        out=at2[:, 4 * g:4 * g + 4, r * P:(r + 1) * P], in_=pt)
            # blocks NPE..NT-1 via the DMA transpose XBAR (SBUF->SBUF bf16)
            for c in range(NPE, NT):
                nc.scalar.dma_start(
                    out=at2[:, c, r * P:(r + 1) * P],
                    in_=a_t[:, c * P:(c + 1) * P], transpose=True)

            # scalar chain for strip r
            d_r = small.tile([P, 1], f32, tag="d")
            nc.vector.tensor_scalar(out=d_r, in0=d_lo, scalar1=d_hi,
                                    scalar2=None, op0=OP.add)
            dsq_r = small.tile([P, 1], f32, tag="dsq")
            nc.scalar.activation(out=dsq_r, in_=d_r, func=AF.Sqrt, bias=eps_sb)
            nc.vector.reciprocal(out=dinv[:, r:r + 1], in_=dsq_r)
            nc.vector.tensor_scalar(out=u0[:, r, :], in0=x_t,
                                    scalar1=dsq_r, scalar2=None, op0=OP.mult)
            nc.vector.tensor_scalar(out=yp0[:, r, :], in0=x_t,
                                    scalar1=dinv[:, r:r + 1], scalar2=None,
                                    op0=OP.mult)

            # PE work that depends on strip r-1's scalar chain, emitted here
            # so that strip r's transposes (ready as soon as the cast lands)
            # sit AHEAD of it in the PE queue -- avoids head-of-line stalls.
            if r >= 1:
                emit_u0T_and_pass1(r - 1)
        emit_u0T_and_pass1(NT - 1)

        # dinv2 = dinv*dinv, all 16 strips in one bulk op
        nc.vector.scalar_tensor_tensor(out=dinv2, in0=dinv, scalar=1.0,
                                       in1=dinv, op0=OP.mult, op1=OP.mult)

        # ---- recurrence: u1T = u0T - z1T (pure DVE, no scaling) --------
        for ch in range(4):
            s = slice(ch * 512, (ch + 1) * 512)
            nc.vector.scalar_tensor_tensor(
                out=uT1[:, s], in0=z1[:, s], scalar=-1.0, in1=uT0[:, s],
                op0=OP.mult, op1=OP.add)

        # ---- pass 2: y1s[c] = dinv2[c]*u1n[c] (scale fused into the
        #      PSUM->SBUF copy, alternating DVE/ACT). The u1n transposes
        #      run two c's ahead so the scale-copy latency stays off the
        #      wide-MM critical path (and the PE never idles long enough
        #      to re-throttle). ------------------------------------------
        z2 = ps_acc.tile([P, N], f32, tag="acc")
        ptcs = {}

        def emit_ptc(c):
            ptc = ps_t.tile([P, 4, P], f32, tag="t")
            nc.tensor.matmul(ptc[:, 0, :], lhsT=uT1[:, c * P:(c + 1) * P],
                             rhs=ident_bf, start=True, stop=True)
            ptcs[c] = ptc

        def emit_y1s(c):
            if c % 2 == 0:
                nc.vector.tensor_scalar(out=y1s[:, c, :], in0=ptcs[c][:, 0, :],
                                        scalar1=dinv2[:, c:c + 1],
                                        scalar2=None, op0=OP.mult)
            else:
                nc.scalar.mul(out=y1s[:, c, :], in_=ptcs[c][:, 0, :],
                              mul=dinv2[:, c:c + 1])

        emit_ptc(0)
        emit_y1s(0)
        emit_ptc(1)
        emit_y1s(1)
        for c in range(NT):
            if c + 2 < NT:
                emit_ptc(c + 2)
                emit_y1s(c + 2)
            for nch in range(4):
                nc.tensor.matmul(z2[:, nch * 512:(nch + 1) * 512],
                                 lhsT=y1s[:, c, :],
                                 rhs=at2[:, c, nch * 512:(nch + 1) * 512],
                                 start=(c == 0), stop=(c == NT - 1))

        # ttT = 2*u1T - u0T (runs on DVE while PE grinds pass 2)
        for ch in range(4):
            s = slice(ch * 512, (ch + 1) * 512)
            nc.vector.scalar_tensor_tensor(
                out=ttT[:, s], in0=uT1[:, s], scalar=2.0, in1=uT0[:, s],
                op0=OP.mult, op1=OP.subtract)

        # ---- output layer: out[i,:] = relu(dinv[i]*(sum_k uTk[:,i]^T Wk)+bsum)
        #      u2T = ttT - 2*z2T produced chunk-by-chunk, interleaved. ----
        uTs = (uT0, uT1, uT2)
        for g in range(4):
            s = slice(g * 512, (g + 1) * 512)
            nc.vector.scalar_tensor_tensor(
                out=uT2[:, s], in0=z2[:, s], scalar=-2.0, in1=ttT[:, s],
                op0=OP.mult, op1=OP.add)
            og = small.tile([P, 4, F], f32, tag="og")
            for q in range(4):
                rr = 4 * g + q
                oc = ps_t.tile([P, 4, P], f32, tag="t")
                for k3 in range(K):
                    nc.tensor.matmul(oc[:, 0, :],
                                     lhsT=uTs[k3][:, rr * P:(rr + 1) * P],
                                     rhs=w_bf[:, k3, :],
                                     start=(k3 == 0), stop=(k3 == K - 1))
                tmp = small.tile([P, F], f32, tag="tmp")
                nc.vector.scalar_tensor_tensor(
                    out=tmp, in0=oc[:, 0, :], scalar=dinv[:, rr:rr + 1],
                    in1=bsum, op0=OP.mult, op1=OP.add)
                nc.scalar.activation(out=og[:, q, :], in_=tmp, func=AF.Relu)
            nc.sync.dma_start(out=out_t[:, 4 * g:4 * g + 4, :], in_=og)

    nc.compile()
    return nc


def _get_nc():
    if "nc" not in _cache:
        _cache["nc"] = _build_nc()
    return _cache["nc"]


def make_in_maps(x, adj, W, b):
    import ml_dtypes

    bf16 = ml_dtypes.bfloat16
    identb = np.ascontiguousarray(np.eye(P, dtype=np.float32).astype(bf16))
    x = np.ascontiguousarray(np.asarray(x, dtype=np.float32))
    adj = np.ascontiguousarray(np.asarray(adj, dtype=np.float32))
    # W [K, in, out] -> [in, K, out] bf16 (host-side rearrange + cast)
    wb = np.ascontiguousarray(
        np.asarray(W, dtype=np.float32).transpose(1, 0, 2).astype(bf16))
    bf = np.asarray(b, dtype=np.float32)
    bsum = np.ascontiguousarray(
        np.broadcast_to(bf.sum(axis=0), (P, F)).astype(np.float32))
    return [
        {"adj": adj[c], "x": x[c], "wb": wb, "bsum": bsum, "identb": identb}
        for c in range(NCORES)
    ]


def run_raw(x, adj, W, b, **kwargs):
    from concourse import bass_utils

    nc = _get_nc()
    in_maps = make_in_maps(x, adj, W, b)
    res = bass_utils.run_bass_kernel_spmd(nc, in_maps,
                                          core_ids=list(range(NCORES)), **kwargs)
    out = np.stack([res.results[c]["out"] for c in range(NCORES)], axis=0)
    return out.astype(np.float32), res


def kernel(x, adj, W, b):
    out, _ = run_raw(x, adj, W, b)
    return out


# revision 26
# speedup vs baseline: 2.9535x; 1.0136x over previous
"""ChebConv (K=3) kernel for Trainium2, data-parallel over batch across 8 NeuronCores.

Math (per batch b):
    d    = adj.sum(axis=1)  (row sums), dinv = (d+eps)^-0.5, dsq = (d+eps)^0.5
    M    = Dinv A Dinv  (L = I - M)
    Tx0 = x, Tx1 = L x, Tx2 = 2 L Tx1 - Tx0
    out  = relu(sum_k Txk @ W[k] + sum_k b[k])

Kernel-side reformulation with u_k := Dsq Tx_k and UNSCALED at2[j, i] = A[i, j]:
every PE pass contracts a Dinv^2-scaled natural operand against plain A^T, so
the transpose of A needs no scaling (it only depends on the bf16 cast, not the
row-sum chain) and the Chebyshev recurrence is elementwise in the transposed
domain:
    u0  = Dsq x,  yp0 = Dinv x  (= Dinv^2 u0)
    z1T = pass(yp0) : z1T[f,i] = sum_j yp0[j,f] at2[j,i]   ( = (Dsq M x)^T )
    u1T = u0T - z1T                                  (DVE subtract)
    y1s[c] = dinv2[c] * u1n[c]   (scale fused into the PSUM->SBUF copy)
    z2T = pass(y1s)                                  ( = (Dsq M Tx1)^T )
    u2T = 2 u1T - u0T - 2 z2T                        (DVE)
    out = relu(Dinv @ (sum_k u_k @ W[k]) + bsum)     (lhsT = u_kT blocks)

A row-strips stream as fp32 in two half-strip DMAs; DVE casts+row-sums the low
half while ACT does the high half (fused accum_out). Transposition of A is
split: blocks c=0..7 on the PE (matmul vs identity, PSUM->SBUF copies on
DVE/ACT), blocks c=8..15 via the DMA transpose XBAR (SBUF->SBUF bf16, scalar
ring) -- this offloads half the pool-engine copy traffic to the otherwise
idle DMA headroom. Pass 1 is emitted triangularly, one strip behind the load
loop, so the PE queue always has cast-ready transpose work at its head.
"""

import numpy as np

B, N, F, K = 8, 2048, 128, 3
P = 128
NT = N // P  # 16
H = N // 2   # 1024
NXB = 0      # A^T blocks per strip produced by the DMA transpose XBAR
EPS = 1e-6
NCORES = 8

_cache = {}


def _build_nc():
    from contextlib import ExitStack

    import concourse.bacc as bacc
    import concourse.tile as tile
    from concourse import mybir

    f32 = mybir.dt.float32
    bf16 = mybir.dt.bfloat16
    AF = mybir.ActivationFunctionType
    OP = mybir.AluOpType

    nc = bacc.Bacc("TRN2", target_bir_lowering=False, debug=False, num_devices=NCORES)
    adj = nc.dram_tensor("adj", [N, N], f32, kind="ExternalInput").ap()
    x = nc.dram_tensor("x", [N, F], f32, kind="ExternalInput").ap()
    wb_d = nc.dram_tensor("wb", [P, K, F], bf16, kind="ExternalInput").ap()
    bsum_d = nc.dram_tensor("bsum", [P, F], f32, kind="ExternalInput").ap()
    identb_d = nc.dram_tensor("identb", [P, P], bf16, kind="ExternalInput").ap()
    out = nc.dram_tensor("out", [N, F], f32, kind="ExternalOutput").ap()
    out_t = out.rearrange("(t p) f -> p t f", p=P)

    NPE = NT - NXB  # PE-transposed blocks per strip (c = 0..NPE-1)

    with ExitStack() as ctx:
        tc = ctx.enter_context(tile.TileContext(nc))
        consts = ctx.enter_context(tc.tile_pool(name="consts", bufs=1))
        afp = ctx.enter_context(tc.tile_pool(name="afp", bufs=6))
        abp = ctx.enter_context(tc.tile_pool(name="abp", bufs=5))
        xfp = ctx.enter_context(tc.tile_pool(name="xfp", bufs=3))
        big = ctx.enter_context(tc.tile_pool(name="big", bufs=1))
        small = ctx.enter_context(tc.tile_pool(name="small", bufs=3))
        ps_acc = ctx.enter_context(tc.tile_pool(name="ps_acc", bufs=1, space="PSUM"))
        ps_t = ctx.enter_context(tc.tile_pool(name="ps_t", bufs=4, space="PSUM"))

        # ---- constants (all HWDGE, scalar ring; sync ring = adj + out) --
        ident_bf = consts.tile([P, P], bf16)
        nc.sync.dma_start(out=ident_bf, in_=identb_d)
        w_bf = consts.tile([P, K, F], bf16)
        nc.sync.dma_start(out=w_bf, in_=wb_d)
        bsum = consts.tile([P, F], f32)
        nc.scalar.dma_start(out=bsum, in_=bsum_d)
        eps_sb = consts.tile([P, 1], f32)
        nc.vector.memset(eps_sb, EPS)

        # per-node scalars, [P, NT]: column r holds values for node tile r
        dinv = consts.tile([P, NT], f32)
        dinv2 = consts.tile([P, NT], f32)

        u0 = big.tile([P, NT, F], bf16)    # Dsq x, natural (u0T source)
        yp0 = big.tile([P, NT, F], bf16)   # Dinv x, natural (pass-1 lhsT)
        y1s = big.tile([P, NT, F], bf16)   # dinv2 * u1, natural (pass-2 lhsT)
        at2 = big.tile([P, NT, N], bf16)   # [j_in_tile, c(j tile), i]: A[i,j]
        uT0 = big.tile([P, N], bf16)       # transposed u0: [f, i]
        uT1 = big.tile([P, N], bf16)
        uT2 = big.tile([P, N], bf16)
        ttT = big.tile([P, N], bf16)       # 2*u1T - u0T

        z1 = ps_acc.tile([P, N], f32, tag="acc")

        def emit_u0T_and_pass1(r):
            # u0T strip r (transpose u0 via PE)
            pt0 = ps_t.tile([P, 4, P], f32, tag="t")
            nc.tensor.matmul(pt0[:, 0, :], lhsT=u0[:, r, :], rhs=ident_bf,
                             start=True, stop=True)
            if r % 2 == 0:
                nc.vector.tensor_copy(out=uT0[:, r * P:(r + 1) * P],
                                      in_=pt0[:, 0, :])
            else:
                nc.scalar.copy(out=uT0[:, r * P:(r + 1) * P], in_=pt0[:, 0, :])
            # triangular pass-1 terms that became ready with strip r:
            # (a) older strips s < r with new weight block c = r (bank chunks)
            for sg in range((r + 3) // 4):
                lo = 4 * sg
                hi = min(lo + 4, r)  # strips [lo, hi)
                nc.tensor.matmul(z1[:, lo * P:hi * P], lhsT=yp0[:, r, :],
                                 rhs=at2[:, r, lo * P:hi * P],
                                 start=False, stop=(r == NT - 1),
                                 skip_group_check=True)
            # (b) strip r, weight blocks c <= r
            for c in range(r + 1):
                nc.tensor.matmul(z1[:, r * P:(r + 1) * P], lhsT=yp0[:, c, :],
                                 rhs=at2[:, c, r * P:(r + 1) * P],
                                 start=(r % 4 == 0 and c == 0),
                                 stop=(c == NT - 1), skip_group_check=True)

        # ---- streaming phase -------------------------------------------
        for r in range(NT):
            a_f = afp.tile([P, N], f32, tag="af")
            nc.sync.dma_start(out=a_f[:, :H], in_=adj[r * P:(r + 1) * P, :H])
            nc.sync.dma_start(out=a_f[:, H:], in_=adj[r * P:(r + 1) * P, H:])
            x_t = xfp.tile([P, F], f32, tag="x")
            nc.scalar.dma_start(out=x_t, in_=x[r * P:(r + 1) * P, :])

            # fused fp32->bf16 cast + row-sum halves (DVE low, ACT high)
            a_t = abp.tile([P, N], bf16, tag="a")
            d_lo = small.tile([P, 1], f32, tag="dlo")
            d_hi = small.tile([P, 1], f32, tag="dhi")
            nc.vector.tensor_scalar(
                out=a_t[:, :H], in0=a_f[:, :H], scalar1=1.0, scalar2=0.0,
                op0=OP.mult, op1=OP.add, accum_out=d_lo)
            nc.scalar.activation(out=a_t[:, H:], in_=a_f[:, H:],
                                 func=AF.Identity, accum_out=d_hi)

            # transpose A strip r, blocks 0..NPE-1 on the PE
            for g in range(NPE // 4):
                pt = ps_t.tile([P, 4, P], f32, tag="t")
                for q in range(4):
                    c = 4 * g + q
                    nc.tensor.matmul(pt[:, q, :], lhsT=a_t[:, c * P:(c + 1) * P],
                                     rhs=ident_bf, start=True, stop=True)
                if g % 2 == 0:
                    nc.vector.tensor_copy(
                        out=at2[:, 4 * g:4 * g + 4, r * P:(r + 1) * P], in_=pt)
                else:
                    nc.scalar.copy(
                        out=at2[:, 4 * g:4 * g + 4, r * P:(r + 1) * P], in_=pt)
            # blocks NPE..NT-1 via the DMA transpose XBAR (SBUF->SBUF bf16)
            for c in range(NPE, NT):
                nc.scalar.dma_start(
                    out=at2[:, c, r * P:(r + 1) * P],
                    in_=a_t[:, c * P:(c + 1) * P], transpose=True)

            # scalar chain for strip r
            d_r = small.tile([P, 1], f32, tag="d")
            nc.vector.tensor_scalar(out=d_r, in0=d_lo, scalar1=d_hi,
                                    scalar2=None, op0=OP.add)
            dsq_r = small.tile([P, 1], f32, tag="dsq")
            nc.scalar.activation(out=dsq_r, in_=d_r, func=AF.Sqrt, bias=eps_sb)
            nc.vector.reciprocal(out=dinv[:, r:r + 1], in_=dsq_r)
            nc.vector.tensor_scalar(out=u0[:, r, :], in0=x_t,
                                    scalar1=dsq_r, scalar2=None, op0=OP.mult)
            nc.vector.tensor_scalar(out=yp0[:, r, :], in0=x_t,
                                    scalar1=dinv[:, r:r + 1], scalar2=None,
                                    op0=OP.mult)

            # PE work that depends on strip r-1's scalar chain, emitted here
            # so that strip r's transposes (ready as soon as the cast lands)
            # sit AHEAD of it in the PE queue -- avoids head-of-line stalls.
            if r >= 1:
                emit_u0T_and_pass1(r - 1)
        emit_u0T_and_pass1(NT - 1)

        # dinv2 = dinv*dinv, all 16 strips in one bulk op
        nc.vector.scalar_tensor_tensor(out=dinv2, in0=dinv, scalar=1.0,
                                       in1=dinv, op0=OP.mult, op1=OP.mult)

        # ---- recurrence: u1T = u0T - z1T (pure DVE, no scaling) --------
        for ch in range(4):
            s = slice(ch * 512, (ch + 1) * 512)
            nc.vector.scalar_tensor_tensor(
                out=uT1[:, s], in0=z1[:, s], scalar=-1.0, in1=uT0[:, s],
                op0=OP.mult, op1=OP.add)

        # ---- pass 2: y1s[c] = dinv2[c]*u1n[c] (scale fused into the
        #      PSUM->SBUF copy, alternating DVE/ACT). The u1n transposes
        #      run two c's ahead so the scale-copy latency stays off the
        #      wide-MM critical path (and the PE never idles long enough
        #      to re-throttle). ------------------------------------------
        z2 = ps_acc.tile([P, N], f32, tag="acc")
        ptcs = {}

        def emit_ptc(c):
            ptc = ps_t.tile([P, 4, P], f32, tag="t")
            nc.tensor.matmul(ptc[:, 0, :], lhsT=uT1[:, c * P:(c + 1) * P],
                             rhs=ident_bf, start=True, stop=True)
            ptcs[c] = ptc

        def emit_y1s(c):
            if c % 2 == 0:
                nc.vector.tensor_scalar(out=y1s[:, c, :], in0=ptcs[c][:, 0, :],
                                        scalar1=dinv2[:, c:c + 1],
                                        scalar2=None, op0=OP.mult)
            else:
                nc.scalar.mul(out=y1s[:, c, :], in_=ptcs[c][:, 0, :],
                              mul=dinv2[:, c:c + 1])

        emit_ptc(0)
        emit_y1s(0)
        emit_ptc(1)
        emit_y1s(1)
        for c in range(NT):
            if c + 2 < NT:
                emit_ptc(c + 2)
                emit_y1s(c + 2)
            for nch in range(4):
                nc.tensor.matmul(z2[:, nch * 512:(nch + 1) * 512],
                                 lhsT=y1s[:, c, :],
                                 rhs=at2[:, c, nch * 512:(nch + 1) * 512],
                                 start=(c == 0), stop=(c == NT - 1))

        # ttT = 2*u1T - u0T (runs on DVE while PE grinds pass 2)
        for ch in range(4):
            s = slice(ch * 512, (ch + 1) * 512)
            nc.vector.scalar_tensor_tensor(
                out=ttT[:, s], in0=uT1[:, s], scalar=2.0, in1=uT0[:, s],
                op0=OP.mult, op1=OP.subtract)

        # ---- output layer: out[i,:] = relu(dinv[i]*(sum_k uTk[:,i]^T Wk)+bsum)
        #      u2T = ttT - 2*z2T produced chunk-by-chunk, interleaved. ----
        uTs = (uT0, uT1, uT2)
        for g in range(4):
            s = slice(g * 512, (g + 1) * 512)
            nc.vector.scalar_tensor_tensor(
                out=uT2[:, s], in0=z2[:, s], scalar=-2.0, in1=ttT[:, s],
                op0=OP.mult, op1=OP.add)
            og = small.tile([P, 4, F], f32, tag="og")
            for q in range(4):
                rr = 4 * g + q
                oc = ps_t.tile([P, 4, P], f32, tag="t")
                for k3 in range(K):
                    nc.tensor.matmul(oc[:, 0, :],
                                     lhsT=uTs[k3][:, rr * P:(rr + 1) * P],
                                     rhs=w_bf[:, k3, :],
                                     start=(k3 == 0), stop=(k3 == K - 1))
                tmp = small.tile([P, F], f32, tag="tmp")
                nc.vector.scalar_tensor_tensor(
                    out=tmp, in0=oc[:, 0, :], scalar=dinv[:, rr:rr + 1],
                    in1=bsum, op0=OP.mult, op1=OP.add)
                nc.scalar.activation(out=og[:, q, :], in_=tmp, func=AF.Relu)
            nc.sync.dma_start(out=out_t[:, 4 * g:4 * g + 4, :], in_=og)

    nc.compile()
    return nc


def _get_nc():
    if "nc" not in _cache:
        _cache["nc"] = _build_nc()
    return _cache["nc"]


def make_in_maps(x, adj, W, b):
    import ml_dtypes

    bf16 = ml_dtypes.bfloat16
    identb = np.ascontiguousarray(np.eye(P, dtype=np.float32).astype(bf16))
    x = np.ascontiguousarray(np.asarray(x, dtype=np.float32))
    adj = np.ascontiguousarray(np.asarray(adj, dtype=np.float32))
    # W [K, in, out] -> [in, K, out] bf16 (host-side rearrange + cast)
    wb = np.ascontiguousarray(
        np.asarray(W, dtype=np.float32).transpose(1, 0, 2).astype(bf16))
    bf = np.asarray(b, dtype=np.float32)
    bsum = np.ascontiguousarray(
        np.broadcast_to(bf.sum(axis=0), (P, F)).astype(np.float32))
    return [
        {"adj": adj[c], "x": x[c], "wb": wb, "bsum": bsum, "identb": identb}
        for c in range(NCORES)
    ]


def run_raw(x, adj, W, b, **kwargs):
    from concourse import bass_utils

    nc = _get_nc()
    in_maps = make_in_maps(x, adj, W, b)
    res = bass_utils.run_bass_kernel_spmd(nc, in_maps,
                                          core_ids=list(range(NCORES)), **kwargs)
    out = np.stack([res.results[c]["out"] for c in range(NCORES)], axis=0)
    return out.astype(np.float32), res


def kernel(x, adj, W, b):
    out, _ = run_raw(x, adj, W, b)
    return out


# revision 27
# speedup vs baseline: 3.0155x; 1.0210x over previous
"""ChebConv (K=3) kernel for Trainium2, data-parallel over batch across 8 NeuronCores.

Math (per batch b):
    d    = adj.sum(axis=1)  (row sums), dinv = (d+eps)^-0.5, dsq = (d+eps)^0.5
    M    = Dinv A Dinv  (L = I - M)
    Tx0 = x, Tx1 = L x, Tx2 = 2 L Tx1 - Tx0
    out  = relu(sum_k Txk @ W[k] + sum_k b[k])

Kernel-side reformulation with u_k := Dsq Tx_k and UNSCALED at2[j, i] = A[i, j]:
every PE pass contracts a Dinv^2-scaled natural operand against plain A^T, so
the transpose of A needs no scaling (it only depends on the bf16 cast, not the
row-sum chain) and the Chebyshev recurrence is elementwise in the transposed
domain:
    u0  = Dsq x,  yp0 = Dinv x  (= Dinv^2 u0)
    z1T = pass(yp0) : z1T[f,i] = sum_j yp0[j,f] at2[j,i]   ( = (Dsq M x)^T )
    u1T = u0T - z1T                                  (DVE subtract)
    y1s[c] = dinv2[c] * u1n[c]   (scale fused into the PSUM->SBUF copy)
    z2T = pass(y1s)                                  ( = (Dsq M Tx1)^T )
    u2T = 2 u1T - u0T - 2 z2T                        (DVE)
    out = relu(Dinv @ (sum_k u_k @ W[k]) + bsum)     (lhsT = u_kT blocks)

A row-strips stream as fp32 in two half-strip DMAs; DVE casts+row-sums the low
half while ACT does the high half (fused accum_out). Transposition of A is
split: blocks c=0..7 on the PE (matmul vs identity, PSUM->SBUF copies on
DVE/ACT), blocks c=8..15 via the DMA transpose XBAR (SBUF->SBUF bf16, scalar
ring) -- this offloads half the pool-engine copy traffic to the otherwise
idle DMA headroom. Pass 1 is emitted triangularly, one strip behind the load
loop, so the PE queue always has cast-ready transpose work at its head.
"""

import numpy as np

B, N, F, K = 8, 2048, 128, 3
P = 128
NT = N // P  # 16
H = N // 2   # 1024
NXB = 0      # A^T blocks per strip produced by the DMA transpose XBAR
EPS = 1e-6
NCORES = 8

_cache = {}


def _build_nc():
    from contextlib import ExitStack

    import concourse.bacc as bacc
    import concourse.tile as tile
    from concourse import mybir

    f32 = mybir.dt.float32
    bf16 = mybir.dt.bfloat16
    AF = mybir.ActivationFunctionType
    OP = mybir.AluOpType

    nc = bacc.Bacc("TRN2", target_bir_lowering=False, debug=False, num_devices=NCORES)
    adj = nc.dram_tensor("adj", [N, N], f32, kind="ExternalInput").ap()
    x = nc.dram_tensor("x", [N, F], f32, kind="ExternalInput").ap()
    wb_d = nc.dram_tensor("wb", [P, K, F], bf16, kind="ExternalInput").ap()
    bsum_d = nc.dram_tensor("bsum", [P, F], f32, kind="ExternalInput").ap()
    identb_d = nc.dram_tensor("identb", [P, P], bf16, kind="ExternalInput").ap()
    out = nc.dram_tensor("out", [N, F], f32, kind="ExternalOutput").ap()
    out_t = out.rearrange("(t p) f -> p t f", p=P)

    NPE = NT - NXB  # PE-transposed blocks per strip (c = 0..NPE-1)

    with ExitStack() as ctx:
        tc = ctx.enter_context(tile.TileContext(nc))
        consts = ctx.enter_context(tc.tile_pool(name="consts", bufs=1))
        afp = ctx.enter_context(tc.tile_pool(name="afp", bufs=7))
        abp = ctx.enter_context(tc.tile_pool(name="abp", bufs=5))
        xfp = ctx.enter_context(tc.tile_pool(name="xfp", bufs=4))
        big = ctx.enter_context(tc.tile_pool(name="big", bufs=1))
        small = ctx.enter_context(tc.tile_pool(name="small", bufs=4))
        ps_acc = ctx.enter_context(tc.tile_pool(name="ps_acc", bufs=1, space="PSUM"))
        ps_t = ctx.enter_context(tc.tile_pool(name="ps_t", bufs=4, space="PSUM"))

        # ---- constants (all HWDGE, scalar ring; sync ring = adj + out) --
        ident_bf = consts.tile([P, P], bf16)
        nc.sync.dma_start(out=ident_bf, in_=identb_d)
        w_bf = consts.tile([P, K, F], bf16)
        nc.sync.dma_start(out=w_bf, in_=wb_d)
        bsum = consts.tile([P, F], f32)
        nc.scalar.dma_start(out=bsum, in_=bsum_d)
        eps_sb = consts.tile([P, 1], f32)
        nc.vector.memset(eps_sb, EPS)

        # per-node scalars, [P, NT]: column r holds values for node tile r
        dinv = consts.tile([P, NT], f32)
        dinv2 = consts.tile([P, NT], f32)

        u0 = big.tile([P, NT, F], bf16)    # Dsq x, natural (u0T source)
        yp0 = big.tile([P, NT, F], bf16)   # Dinv x, natural (pass-1 lhsT)
        y1s = big.tile([P, NT, F], bf16)   # dinv2 * u1, natural (pass-2 lhsT)
        at2 = big.tile([P, NT, N], bf16)   # [j_in_tile, c(j tile), i]: A[i,j]
        uT0 = big.tile([P, N], bf16)       # transposed u0: [f, i]
        uT1 = big.tile([P, N], bf16)
        uT2 = big.tile([P, N], bf16)
        ttT = big.tile([P, N], bf16)       # 2*u1T - u0T

        z1 = ps_acc.tile([P, N], f32, tag="acc")

        def emit_u0T_and_pass1(r):
            # u0T strip r (transpose u0 via PE)
            pt0 = ps_t.tile([P, 4, P], f32, tag="t")
            nc.tensor.matmul(pt0[:, 0, :], lhsT=u0[:, r, :], rhs=ident_bf,
                             start=True, stop=True)
            if r % 2 == 0:
                nc.vector.tensor_copy(out=uT0[:, r * P:(r + 1) * P],
                                      in_=pt0[:, 0, :])
            else:
                nc.scalar.copy(out=uT0[:, r * P:(r + 1) * P], in_=pt0[:, 0, :])
            # triangular pass-1 terms that became ready with strip r:
            # (a) older strips s < r with new weight block c = r (bank chunks)
            for sg in range((r + 3) // 4):
                lo = 4 * sg
                hi = min(lo + 4, r)  # strips [lo, hi)
                nc.tensor.matmul(z1[:, lo * P:hi * P], lhsT=yp0[:, r, :],
                                 rhs=at2[:, r, lo * P:hi * P],
                                 start=False, stop=(r == NT - 1),
                                 skip_group_check=True)
            # (b) strip r, weight blocks c <= r
            for c in range(r + 1):
                nc.tensor.matmul(z1[:, r * P:(r + 1) * P], lhsT=yp0[:, c, :],
                                 rhs=at2[:, c, r * P:(r + 1) * P],
                                 start=(r % 4 == 0 and c == 0),
                                 stop=(c == NT - 1), skip_group_check=True)

        # ---- streaming phase -------------------------------------------
        for r in range(NT):
            a_f = afp.tile([P, N], f32, tag="af")
            nc.sync.dma_start(out=a_f[:, :H], in_=adj[r * P:(r + 1) * P, :H])
            nc.sync.dma_start(out=a_f[:, H:], in_=adj[r * P:(r + 1) * P, H:])
            x_t = xfp.tile([P, F], f32, tag="x")
            nc.scalar.dma_start(out=x_t, in_=x[r * P:(r + 1) * P, :])

            # fused fp32->bf16 cast + row-sum halves (DVE low, ACT high)
            a_t = abp.tile([P, N], bf16, tag="a")
            d_lo = small.tile([P, 1], f32, tag="dlo")
            d_hi = small.tile([P, 1], f32, tag="dhi")
            nc.vector.tensor_scalar(
                out=a_t[:, :H], in0=a_f[:, :H], scalar1=1.0, scalar2=0.0,
                op0=OP.mult, op1=OP.add, accum_out=d_lo)
            nc.scalar.activation(out=a_t[:, H:], in_=a_f[:, H:],
                                 func=AF.Identity, accum_out=d_hi)

            # transpose A strip r, blocks 0..NPE-1 on the PE
            for g in range(NPE // 4):
                pt = ps_t.tile([P, 4, P], f32, tag="t")
                for q in range(4):
                    c = 4 * g + q
                    nc.tensor.matmul(pt[:, q, :], lhsT=a_t[:, c * P:(c + 1) * P],
                                     rhs=ident_bf, start=True, stop=True)
                if g % 2 == 0:
                    nc.vector.tensor_copy(
                        out=at2[:, 4 * g:4 * g + 4, r * P:(r + 1) * P], in_=pt)
                else:
                    nc.scalar.copy(
                        out=at2[:, 4 * g:4 * g + 4, r * P:(r + 1) * P], in_=pt)
            # blocks NPE..NT-1 via the DMA transpose XBAR (SBUF->SBUF bf16)
            for c in range(NPE, NT):
                nc.scalar.dma_start(
                    out=at2[:, c, r * P:(r + 1) * P],
                    in_=a_t[:, c * P:(c + 1) * P], transpose=True)

            # scalar chain for strip r
            d_r = small.tile([P, 1], f32, tag="d")
            nc.vector.tensor_scalar(out=d_r, in0=d_lo, scalar1=d_hi,
                                    scalar2=None, op0=OP.add)
            dsq_r = small.tile([P, 1], f32, tag="dsq")
            nc.scalar.activation(out=dsq_r, in_=d_r, func=AF.Sqrt, bias=eps_sb)
            nc.vector.reciprocal(out=dinv[:, r:r + 1], in_=dsq_r)
            nc.vector.tensor_scalar(out=u0[:, r, :], in0=x_t,
                                    scalar1=dsq_r, scalar2=None, op0=OP.mult)
            nc.vector.tensor_scalar(out=yp0[:, r, :], in0=x_t,
                                    scalar1=dinv[:, r:r + 1], scalar2=None,
                                    op0=OP.mult)

            # PE work that depends on strip r-1's scalar chain, emitted here
            # so that strip r's transposes (ready as soon as the cast lands)
            # sit AHEAD of it in the PE queue -- avoids head-of-line stalls.
            if r >= 1:
                emit_u0T_and_pass1(r - 1)
        emit_u0T_and_pass1(NT - 1)

        # dinv2 = dinv*dinv, all 16 strips in one bulk op
        nc.vector.scalar_tensor_tensor(out=dinv2, in0=dinv, scalar=1.0,
                                       in1=dinv, op0=OP.mult, op1=OP.mult)

        # ---- recurrence: u1T = u0T - z1T (pure DVE, no scaling) --------
        for ch in range(4):
            s = slice(ch * 512, (ch + 1) * 512)
            nc.vector.scalar_tensor_tensor(
                out=uT1[:, s], in0=z1[:, s], scalar=-1.0, in1=uT0[:, s],
                op0=OP.mult, op1=OP.add)

        # ---- pass 2: y1s[c] = dinv2[c]*u1n[c] (scale fused into the
        #      PSUM->SBUF copy, alternating DVE/ACT). The u1n transposes
        #      run two c's ahead so the scale-copy latency stays off the
        #      wide-MM critical path (and the PE never idles long enough
        #      to re-throttle). ------------------------------------------
        z2 = ps_acc.tile([P, N], f32, tag="acc")
        ptcs = {}

        def emit_ptc(c):
            ptc = ps_t.tile([P, 4, P], f32, tag="t")
            nc.tensor.matmul(ptc[:, 0, :], lhsT=uT1[:, c * P:(c + 1) * P],
                             rhs=ident_bf, start=True, stop=True)
            ptcs[c] = ptc

        def emit_y1s(c):
            if c % 2 == 0:
                nc.vector.tensor_scalar(out=y1s[:, c, :], in0=ptcs[c][:, 0, :],
                                        scalar1=dinv2[:, c:c + 1],
                                        scalar2=None, op0=OP.mult)
            else:
                nc.scalar.mul(out=y1s[:, c, :], in_=ptcs[c][:, 0, :],
                              mul=dinv2[:, c:c + 1])

        emit_ptc(0)
        emit_y1s(0)
        emit_ptc(1)
        emit_y1s(1)
        for c in range(NT):
            if c + 2 < NT:
                emit_ptc(c + 2)
                emit_y1s(c + 2)
            for nch in range(4):
                nc.tensor.matmul(z2[:, nch * 512:(nch + 1) * 512],
                                 lhsT=y1s[:, c, :],
                                 rhs=at2[:, c, nch * 512:(nch + 1) * 512],
                                 start=(c == 0), stop=(c == NT - 1))

        # ttT = 2*u1T - u0T (runs on DVE while PE grinds pass 2)
        for ch in range(4):
            s = slice(ch * 512, (ch + 1) * 512)
            nc.vector.scalar_tensor_tensor(
                out=ttT[:, s], in0=uT1[:, s], scalar=2.0, in1=uT0[:, s],
                op0=OP.mult, op1=OP.subtract)

        # ---- output layer: out[i,:] = relu(dinv[i]*(sum_k uTk[:,i]^T Wk)+bsum)
        #      u2T = ttT - 2*z2T produced chunk-by-chunk, interleaved. ----
        uTs = (uT0, uT1, uT2)
        for g in range(4):
            s = slice(g * 512, (g + 1) * 512)
            nc.vector.scalar_tensor_tensor(
                out=uT2[:, s], in0=z2[:, s], scalar=-2.0, in1=ttT[:, s],
                op0=OP.mult, op1=OP.add)
            og = small.tile([P, 4, F], f32, tag="og")
            for q in range(4):
                rr = 4 * g + q
                oc = ps_t.tile([P, 4, P], f32, tag="t")
                for k3 in range(K):
                    nc.tensor.matmul(oc[:, 0, :],
                                     lhsT=uTs[k3][:, rr * P:(rr + 1) * P],
                                     rhs=w_bf[:, k3, :],
                                     start=(k3 == 0), stop=(k3 == K - 1))
                tmp = small.tile([P, F], f32, tag="tmp")
                nc.vector.scalar_tensor_tensor(
                    out=tmp, in0=oc[:, 0, :], scalar=dinv[:, rr:rr + 1],
                    in1=bsum, op0=OP.mult, op1=OP.add)
                nc.scalar.activation(out=og[:, q, :], in_=tmp, func=AF.Relu)
            nc.sync.dma_start(out=out_t[:, 4 * g:4 * g + 4, :], in_=og)

    nc.compile()
    return nc


def _get_nc():
    if "nc" not in _cache:
        _cache["nc"] = _build_nc()
    return _cache["nc"]


def make_in_maps(x, adj, W, b):
    import ml_dtypes

    bf16 = ml_dtypes.bfloat16
    identb = np.ascontiguousarray(np.eye(P, dtype=np.float32).astype(bf16))
    x = np.ascontiguousarray(np.asarray(x, dtype=np.float32))
    adj = np.ascontiguousarray(np.asarray(adj, dtype=np.float32))
    # W [K, in, out] -> [in, K, out] bf16 (host-side rearrange + cast)
    wb = np.ascontiguousarray(
        np.asarray(W, dtype=np.float32).transpose(1, 0, 2).astype(bf16))
    bf = np.asarray(b, dtype=np.float32)
    bsum = np.ascontiguousarray(
        np.broadcast_to(bf.sum(axis=0), (P, F)).astype(np.float32))
    return [
        {"adj": adj[c], "x": x[c], "wb": wb, "bsum": bsum, "identb": identb}
        for c in range(NCORES)
    ]


def run_raw(x, adj, W, b, **kwargs):
    from concourse import bass_utils

    nc = _get_nc()
    in_maps = make_in_maps(x, adj, W, b)
    res = bass_utils.run_bass_kernel_spmd(nc, in_maps,
                                          core_ids=list(range(NCORES)), **kwargs)
    out = np.stack([res.results[c]["out"] for c in range(NCORES)], axis=0)
    return out.astype(np.float32), res


def kernel(x, adj, W, b):
    out, _ = run_raw(x, adj, W, b)
    return out
